# revision 3
# baseline (speedup 1.0000x reference)
"""Trainium2 Bass kernel for nn_AttentionDecoder, v3: 8-core token-split.

Each batch element (B=4) is handled by a PAIR of NeuronCores; each core owns
512 of the 1024 tokens.  Per layer, the pair exchanges bf16 copies of their
updated residual halves with one ncfw AllGather (~0.75 MB/rank), overlapped
with the own-half LN/Q/K/V compute.  Keys/values cover all 1024 tokens in
"own-first" order (attention is key-permutation invariant), so both cores run
the IDENTICAL program: partner-block selection out of the AllGather result is
data-driven via per-core 0/1 mask inputs, not parity-specialized code.
Everything else follows v2: bf16 weights + activations, chunked weight DMAs,
token-major V, softmax denominator fused into attn@V via an augmented ones
column, row-tiled concurrent head-pair score matmuls.
"""

import os
import sys

import numpy as np

for _p in ("/opt/trn_rl_repo", "/opt/pypackages"):
    if _p not in sys.path:
        sys.path.append(_p)

B = 4
NCORES = 8
F_DIM = 256
H = W = 32
NT = H * W          # 1024 keys (all tokens)
NTO = NT // 2       # 512 owned tokens per core
DIM = 768
DEPTH = 8
HEADS = 12
DH = DIM // HEADS
MLP = 3072
SCALE = DH ** -0.5
LN_EPS = 1e-5

P = 128
FC = DIM // P
TC = NT // P        # 8 key chunks
TCO = NTO // P      # 4 owned token chunks
MC = MLP // P
NH = NT // 2        # 512 query count (the owned half)
PAIRS = HEADS // 2
VCOL = DH + 1

_CACHE = {}


def _sine_pos_embed(h, w, num_pos_feats):
    scale = 2.0 * np.pi
    eps = 1e-6
    y = np.arange(1, h + 1, dtype=np.float32) / np.float32(h + eps) * np.float32(scale)
    x = np.arange(1, w + 1, dtype=np.float32) / np.float32(w + eps) * np.float32(scale)
    i = np.arange(num_pos_feats, dtype=np.float32)
    dim_t = (10000.0 ** (2.0 * np.floor(i / 2.0) / num_pos_feats)).astype(np.float32)

    def interleave(p):
        return np.stack(
            [np.sin(p[..., 0::2]), np.cos(p[..., 1::2])], axis=-1
        ).reshape(p.shape[:-1] + (-1,))

    pos_y = interleave((y[:, None] / dim_t).astype(np.float32))
    pos_x = interleave((x[:, None] / dim_t).astype(np.float32))
    pos = np.concatenate(
        [
            np.broadcast_to(pos_y[:, None, :], (h, w, num_pos_feats)),
            np.broadcast_to(pos_x[None, :, :], (h, w, num_pos_feats)),
        ],
        axis=-1,
    )
    return pos.reshape(h * w, 2 * num_pos_feats).astype(np.float32)


def _build_program(depth=DEPTH):
    import concourse.bass as bass
    import concourse.mybir as mybir
    import concourse.tile as tile
    from concourse import bacc

    f32 = mybir.dt.float32
    f32r = mybir.dt.float32r
    bf16 = mybir.dt.bfloat16
    AF = mybir.ActivationFunctionType
    RGS = [[2 * i, 2 * i + 1] for i in range(NCORES // 2)]

    nc = bacc.Bacc(
        "TRN2",
        target_bir_lowering=False,
        debug=False,
        enable_asserts=False,
        num_devices=NCORES,
    )

    cf = nc.dram_tensor("cf", [F_DIM, NTO], f32, kind="ExternalInput").ap()
    posT = nc.dram_tensor("posT", [DIM, NTO], f32, kind="ExternalInput").ap()
    cwT = nc.dram_tensor("cwT", [F_DIM, DIM], f32, kind="ExternalInput").ap()
    qkvw = nc.dram_tensor("qkvw", [DEPTH, DIM, 3 * DIM], bf16, kind="ExternalInput").ap()
    outw = nc.dram_tensor("outw", [DEPTH, DIM, DIM], bf16, kind="ExternalInput").ap()
    w1 = nc.dram_tensor("w1", [DEPTH, DIM, MLP], bf16, kind="ExternalInput").ap()
    w2 = nc.dram_tensor("w2", [DEPTH, MLP, DIM], bf16, kind="ExternalInput").ap()
    msel = nc.dram_tensor("msel", [P, 2], f32, kind="ExternalInput").ap()
    out = nc.dram_tensor("out", [DIM, NTO], f32, kind="ExternalOutput").ap()

    r = lambda ap: ap.bitcast(f32r)

    with tile.TileContext(nc) as tc:
        from contextlib import ExitStack

        with ExitStack() as ctx:
            ctx.enter_context(
                nc.allow_low_precision(reason="bf16 weights/activations")
            )
            const = ctx.enter_context(tc.tile_pool(name="const", bufs=1))
            bigx = ctx.enter_context(tc.tile_pool(name="bigx", bufs=1))
            ybuf = ctx.enter_context(tc.tile_pool(name="ybuf", bufs=1))
            qkbuf = ctx.enter_context(tc.tile_pool(name="qkbuf", bufs=1))
            vtbuf = ctx.enter_context(tc.tile_pool(name="vtbuf", bufs=1))
            esbuf = ctx.enter_context(tc.tile_pool(name="esbuf", bufs=1))
            obuf = ctx.enter_context(tc.tile_pool(name="obuf", bufs=1))
            hbuf = ctx.enter_context(tc.tile_pool(name="hbuf", bufs=1))
            xchg = ctx.enter_context(tc.tile_pool(name="xchg", bufs=1))
            sqp = ctx.enter_context(tc.tile_pool(name="sqp", bufs=3))
            wch = ctx.enter_context(tc.tile_pool(name="wch", bufs=2))
            w2p = ctx.enter_context(tc.tile_pool(name="w2p", bufs=1))
            lines = ctx.enter_context(tc.tile_pool(name="lines", bufs=5))
            rbsp = ctx.enter_context(tc.tile_pool(name="rbsp", bufs=2))
            dram = ctx.enter_context(tc.tile_pool(name="dram", bufs=2, space="DRAM"))
            ps = ctx.enter_context(tc.tile_pool(name="ps", bufs=4, space="PSUM"))
            oacp = ctx.enter_context(tc.tile_pool(name="oacp", bufs=2, space="PSUM"))
            lnp = ctx.enter_context(tc.tile_pool(name="lnp", bufs=2, space="PSUM"))

            ones_stage = const.tile([P, P], f32, tag="ones_stage")
            nc.gpsimd.memset(ones_stage[:], 1.0)
            ones_col = const.tile([P, 1], f32, tag="ones_col")
            nc.vector.tensor_copy(r(ones_col[:]), ones_stage[:, 0:1])
            ones_row = const.tile([1, P], f32, tag="ones_row")
            nc.vector.tensor_copy(r(ones_row[:]), ones_stage[0:1, :])
            ones_col_b = const.tile([P, 1], bf16, tag="ones_col_b")
            nc.vector.tensor_copy(ones_col_b[:], ones_stage[:, 0:1])
            msel_sb = const.tile([P, 2], f32, tag="msel_sb")
            nc.sync.dma_start(msel_sb[:], msel[:])

            # residual stream for the OWN 512 tokens, feature-major, fp32
            x = bigx.tile([P, FC, NTO], f32, tag="x")

            # ---- conv (1x1) + positional embedding (own half, fp32) ----
            cf_sb = hbuf.tile([P, 2, NTO], f32, tag="h", name="cfsb")
            nc.sync.dma_start(r(cf_sb[:]), r(cf.rearrange("(c p) t -> p c t", p=P)))
            pos_sb = qkbuf.tile([P, FC, NTO], f32, tag="qk", name="possb")
            nc.sync.dma_start(pos_sb[:], posT.rearrange("(c p) t -> p c t", p=P))
            cw_sb = w2p.tile([P, 2, DIM], f32, tag="w2", name="cwsb")
            nc.sync.dma_start(r(cw_sb[:]), r(cwT.rearrange("(c p) m -> p c m", p=P)))
            for m in range(FC):
                pt = ps.tile([P, NH], f32, tag="mm")
                for k in range(2):
                    nc.tensor.matmul(
                        pt[:], r(cw_sb[:, k, m * P:(m + 1) * P]),
                        r(cf_sb[:, k, :]),
                        start=(k == 0), stop=(k == 1),
                    )
                nc.vector.tensor_add(
                    r(x[:, m, :]), pt[:], pos_sb[:, m, :],
                )

            # persistent buffers
            y = ybuf.tile([P, FC, NT], bf16, tag="y")       # own-first token order
            qk = qkbuf.tile([P, 2 * PAIRS, NT], bf16, tag="qk", name="qktile")
            vT = vtbuf.tile([P, TC, HEADS * VCOL], bf16, tag="vT")
            es = esbuf.tile([P, 2, TC, NH], bf16, tag="es")
            o_sb = obuf.tile([P, FC, NTO], bf16, tag="o")

            for hd in range(HEADS):
                nc.vector.memset(vT[:, :, hd * VCOL + DH:hd * VCOL + DH + 1], 1.0)

            def ln_own(xin, yout, coff):
                """LN of own-half fp32 residual -> yout[:, :, coff:coff+NTO]."""
                s_ps = lnp.tile([1, NTO], f32, tag="ln", name="sps_f")
                q_ps = lnp.tile([1, NTO], f32, tag="ln", name="qps_f")
                for c in range(FC):
                    sq = sqp.tile([P, NTO], f32, tag="sq")
                    nc.vector.tensor_mul(r(sq[:]), xin[:, c, :], xin[:, c, :])
                    nc.tensor.matmul(
                        s_ps[:], r(ones_col[:]), r(xin[:, c, :]),
                        start=(c == 0), stop=(c == FC - 1),
                    )
                    nc.tensor.matmul(
                        q_ps[:], r(ones_col[:]), r(sq[:]),
                        start=(c == 0), stop=(c == FC - 1),
                    )
                _ln_tail(xin, yout, coff, s_ps, q_ps, None)

            def ln_other(xgo, yout, coff):
                """LN of partner-half bf16 residual -> yout[:, :, coff:+NTO]."""
                s_ps = lnp.tile([1, NTO], f32, tag="ln", name="sps_b")
                q_ps = lnp.tile([1, NTO], f32, tag="ln", name="qps_b")
                for c in range(FC):
                    sqb = sqp.tile([P, NTO], bf16, tag="sqb")
                    nc.vector.tensor_mul(sqb[:], xgo[:, c, :], xgo[:, c, :])
                    nc.tensor.matmul(
                        s_ps[:], ones_col_b[:], xgo[:, c, :],
                        start=(c == 0), stop=(c == FC - 1),
                    )
                    nc.tensor.matmul(
                        q_ps[:], ones_col_b[:], sqb[:],
                        start=(c == 0), stop=(c == FC - 1),
                    )
                _ln_tail(xgo, yout, coff, s_ps, q_ps, None)

            def _ln_tail(xin, yout, coff, s_ps, q_ps, _):
                mean = lines.tile([1, NTO], f32, tag="lnl", name="mean")
                nc.vector.tensor_scalar_mul(mean[:], s_ps[:], 1.0 / DIM)
                msq = lines.tile([1, NTO], f32, tag="lnl", name="msq")
                nc.vector.tensor_mul(msq[:], mean[:], mean[:])
                var = lines.tile([1, NTO], f32, tag="lnl", name="var")
                nc.vector.tensor_scalar(
                    var[:], q_ps[:], 1.0 / DIM, LN_EPS,
                    mybir.AluOpType.mult, mybir.AluOpType.add,
                )
                nc.vector.tensor_sub(var[:], var[:], msq[:])
                lnv = lines.tile([1, NTO], f32, tag="lnl", name="lnv")
                nc.scalar.activation(lnv[:], var[:], AF.Ln, bias=0.0, scale=1.0)
                a = lines.tile([1, NTO], f32, tag="lnl", name="a")
                nc.scalar.activation(r(a[:]), lnv[:], AF.Exp, bias=0.0, scale=-0.5)
                cl = lines.tile([1, NTO], f32, tag="lnl", name="cl")
                nc.vector.tensor_mul(r(cl[:]), mean[:], a[:])
                ab = ps.tile([P, NTO], f32, tag="mm")
                cb = ps.tile([P, NTO], f32, tag="mm")
                nc.tensor.matmul(ab[:], r(ones_row[:]), r(a[:]))
                nc.tensor.matmul(cb[:], r(ones_row[:]), r(cl[:]))
                for c in range(FC):
                    tmp = sqp.tile([P, NTO], f32, tag="sq", name="lntmp")
                    nc.vector.tensor_mul(r(tmp[:]), xin[:, c, :], ab[:])
                    nc.vector.tensor_sub(yout[:, c, coff:coff + NTO], tmp[:], cb[:])

            def load_chunk(src_l, j, name):
                wt = wch.tile([P, FC, FC * P], bf16, tag="wc", name=name)
                nc.sync.dma_start(
                    wt[:],
                    src_l[:, j * FC * P:(j + 1) * FC * P].rearrange(
                        "(c p) m -> p c m", p=P
                    ),
                )
                return wt

            def qk_proj(wt, j, tok0, ntok):
                """d-major projection of y[:, :, tok0:tok0+ntok] into qk."""
                for m in range(FC):
                    pt = ps.tile([P, NH], f32, tag="mm", name="qkpt")
                    for k in range(FC):
                        nc.tensor.matmul(
                            pt[:, 0:ntok], wt[:, k, m * P:(m + 1) * P],
                            y[:, k, tok0:tok0 + ntok],
                            start=(k == 0), stop=(k == FC - 1),
                        )
                    nc.vector.tensor_copy(
                        qk[:, j * PAIRS + m, tok0:tok0 + ntok], pt[:, 0:ntok]
                    )

            def v_proj(wv, tclo, tchi):
                """token-major V for token chunks [tclo, tchi)."""
                for g in range(2):
                    for t in range(tclo, tchi):
                        vp = ps.tile([P, 3 * P], f32, tag="mm", name="vps")
                        for c in range(FC):
                            nc.tensor.matmul(
                                vp[:], y[:, c, t * P:(t + 1) * P],
                                wv[:, c, g * 3 * P:(g + 1) * 3 * P],
                                start=(c == 0), stop=(c == FC - 1),
                            )
                        for hl in range(PAIRS):
                            hd = g * PAIRS + hl
                            nc.vector.tensor_copy(
                                vT[:, t, hd * VCOL:hd * VCOL + DH],
                                vp[:, hl * DH:(hl + 1) * DH],
                            )

            for l in range(depth):
                # ---- pair exchange of the fp32->bf16 residual half ----
                xb = xchg.tile([P, FC, NTO], bf16, tag="xb", name="xb")
                nc.vector.tensor_copy(xb[:], x[:])
                ib = dram.tile([P, FC, NTO], bf16, tag="ib", name="ib")
                nc.gpsimd.dma_start(ib[:], xb[:])
                ob = dram.tile([2 * P, FC, NTO], bf16, tag="ob", name="ob")
                nc.gpsimd.collective_compute(
                    "AllGather", mybir.AluOpType.bypass, replica_groups=RGS,
                    ins=[ib.opt()], outs=[ob.opt()],
                )
                t2 = xchg.tile([P, 2, FC, NTO], bf16, tag="t2", name="t2")
                nc.gpsimd.dma_start(t2[:, 0, :, :], ob[0:P, :, :])
                nc.gpsimd.dma_start(t2[:, 1, :, :], ob[P:2 * P, :, :])
                xgo = xchg.tile([P, FC, NTO], bf16, tag="xgo", name="xgo")
                nc.vector.tensor_scalar_mul(xgo[:], t2[:, 0, :, :], msel_sb[:, 0:1])
                nc.vector.scalar_tensor_tensor(
                    xgo[:], t2[:, 1, :, :], msel_sb[:, 1:2], xgo[:],
                    mybir.AluOpType.mult, mybir.AluOpType.add,
                )

                # ---- own-half LN + Q + K(own cols) + V(own rows) ----
                ln_own(x, y, 0)
                wq = load_chunk(qkvw[l], 0, "wq")
                qk_proj(wq, 0, 0, NTO)        # Q for own queries
                wk = load_chunk(qkvw[l], 1, "wk")
                qk_proj(wk, 1, 0, NTO)        # K own columns
                wv = load_chunk(qkvw[l], 2, "wv")
                v_proj(wv, 0, TCO)            # V own rows

                # ---- partner-half LN + K + V (waits on the AllGather) ----
                ln_other(xgo, y, NTO)
                qk_proj(wk, 1, NTO, NTO)      # K partner columns
                v_proj(wv, TCO, TC)           # V partner rows

                # ---- attention (own 512 queries, all 1024 keys) ----
                for hp in range(PAIRS):
                    oas = [oacp.tile([VCOL, NH], f32, tag="oac", name="oas")
                           for _ in range(2)]
                    for kc in range(TC):
                        sps = [ps.tile([P, NH], f32, tag="mm", name="sps")
                               for _ in range(2)]
                        for hh in range(2):
                            b0 = DH * hh
                            nc.tensor.matmul(
                                sps[hh][:],
                                qk[b0:b0 + DH, PAIRS + hp, kc * P:(kc + 1) * P],
                                qk[b0:b0 + DH, hp, 0:NTO],
                            )
                        for hh in range(2):
                            nc.scalar.activation(
                                es[:, hh, kc, :], sps[hh][:],
                                AF.Exp, bias=0.0, scale=SCALE,
                            )
                    for kc in range(TC):
                        for hh in range(2):
                            hd = hp * 2 + hh
                            nc.tensor.matmul(
                                oas[hh][:],
                                vT[:, kc, hd * VCOL:(hd + 1) * VCOL],
                                es[:, hh, kc, :],
                                start=(kc == 0), stop=(kc == TC - 1),
                            )
                    for hh in range(2):
                        rl = lines.tile([1, NH], f32, tag="lnl", name="rl")
                        nc.scalar.activation(
                            r(rl[:]), oas[hh][DH:DH + 1, :],
                            AF.Ln, bias=0.0, scale=1.0,
                        )
                        rb = ps.tile([P, NH], f32, tag="mm")
                        nc.tensor.matmul(rb[:DH, :], r(ones_row[:, 0:DH]), r(rl[:]))
                        rbs = rbsp.tile([DH, NH], bf16, tag="rbs")
                        nc.scalar.activation(
                            rbs[:], rb[:DH, :], AF.Exp, bias=0.0, scale=-1.0,
                        )
                        nc.vector.tensor_mul(
                            o_sb[hh * DH:(hh + 1) * DH, hp, :],
                            oas[hh][:DH, :], rbs[:],
                        )

                # ---- out projection + residual (own half) ----
                wo = load_chunk(outw[l], 0, "wo")
                for m in range(FC):
                    pt = ps.tile([P, NH], f32, tag="mm", name="opt")
                    for k in range(FC):
                        nc.tensor.matmul(
                            pt[:], wo[:, k, m * P:(m + 1) * P], o_sb[:, k, :],
                            start=(k == 0), stop=(k == FC - 1),
                        )
                    nc.vector.tensor_add(r(x[:, m, :]), x[:, m, :], pt[:])

                # ---- MLP (own half) ----
                ln_own(x, y, 0)
                w2t = w2p.tile([P, MC, DIM], bf16, tag="w2", name="w2t")
                nc.sync.dma_start(w2t[:], w2[l].rearrange("(c p) m -> p c m", p=P))
                hs = hbuf.tile([P, MC, NTO], bf16, tag="h", name="hs")
                for j in range(4):
                    wt = load_chunk(w1[l], j, "w1")
                    for i in range(FC):
                        mi = j * FC + i
                        pt = ps.tile([P, NH], f32, tag="mm", name="mpt")
                        for k in range(FC):
                            nc.tensor.matmul(
                                pt[:], wt[:, k, i * P:(i + 1) * P], y[:, k, 0:NTO],
                                start=(k == 0), stop=(k == FC - 1),
                            )
                        nc.scalar.activation(
                            hs[:, mi, :], pt[:], AF.Gelu, bias=0.0, scale=1.0
                        )
                for m in range(FC):
                    pt = ps.tile([P, NH], f32, tag="mm", name="m2pt")
                    for i in range(MC):
                        nc.tensor.matmul(
                            pt[:], w2t[:, i, m * P:(m + 1) * P], hs[:, i, :],
                            start=(i == 0), stop=(i == MC - 1),
                        )
                    nc.vector.tensor_add(r(x[:, m, :]), x[:, m, :], pt[:])

            nc.sync.dma_start(out.rearrange("(c p) t -> p c t", p=P), x[:])

    nc.finalize()
    return nc


def _prepare(inputs):
    import ml_dtypes

    bf16 = ml_dtypes.bfloat16

    c_f = np.ascontiguousarray(inputs["c_f"], dtype=np.float32)
    conv_w = np.asarray(inputs["conv_w"], dtype=np.float32)
    conv_b = np.asarray(inputs["conv_b"], dtype=np.float32)
    ln1_w = np.asarray(inputs["ln1_w"], dtype=np.float32)
    ln1_b = np.asarray(inputs["ln1_b"], dtype=np.float32)
    qkv_w = np.asarray(inputs["qkv_w"], dtype=np.float32)
    out_w = np.asarray(inputs["out_w"], dtype=np.float32)
    out_b = np.asarray(inputs["out_b"], dtype=np.float32)
    ln2_w = np.asarray(inputs["ln2_w"], dtype=np.float32)
    ln2_b = np.asarray(inputs["ln2_b"], dtype=np.float32)
    mlp_w1 = np.asarray(inputs["mlp_w1"], dtype=np.float32)
    mlp_b1 = np.asarray(inputs["mlp_b1"], dtype=np.float32)
    mlp_w2 = np.asarray(inputs["mlp_w2"], dtype=np.float32)
    mlp_b2 = np.asarray(inputs["mlp_b2"], dtype=np.float32)

    qkv_b_eff = np.einsum("ld,ldm->lm", ln1_b, qkv_w)
    b1_eff = np.einsum("ld,ldm->lm", ln2_b, mlp_w1) + mlp_b1
    assert not np.any(qkv_b_eff), "nonzero effective qkv bias unsupported"
    assert not np.any(out_b), "nonzero out bias unsupported"
    assert not np.any(b1_eff), "nonzero effective mlp bias unsupported"
    assert not np.any(mlp_b2), "nonzero mlp_b2 unsupported"

    pos = _sine_pos_embed(H, W, DIM // 2)            # [1024, 768]
    posT = np.ascontiguousarray(pos.T + conv_b[:, None]).astype(np.float32)
    cwT = np.ascontiguousarray(conv_w.T).astype(np.float32)

    qkvw_eff = np.ascontiguousarray(ln1_w[:, :, None] * qkv_w).astype(bf16)
    w1_eff = np.ascontiguousarray(ln2_w[:, :, None] * mlp_w1).astype(bf16)

    shared = {
        "cwT": cwT,
        "qkvw": qkvw_eff,
        "outw": np.ascontiguousarray(out_w).astype(bf16),
        "w1": w1_eff,
        "w2": np.ascontiguousarray(mlp_w2).astype(bf16),
    }

    cf_all = c_f.reshape(B, F_DIM, NT)
    in_maps = []
    for c in range(NCORES):
        b, hf = c // 2, c % 2
        m = np.zeros((P, 2), np.float32)
        m[:, hf] = 1.0  # select partner block: block (1-hf)... see below
        # partner block index = 1 - hf is the OTHER rank's position in the
        # gather; rank parity hf sits at block hf, partner at block 1-hf.
        m[:, :] = 0.0
        m[:, 1 - hf] = 1.0
        in_maps.append(dict(
            shared,
            cf=np.ascontiguousarray(cf_all[b][:, hf * NTO:(hf + 1) * NTO]),
            posT=np.ascontiguousarray(posT[:, hf * NTO:(hf + 1) * NTO]),
            msel=m,
        ))
    return in_maps


class _Runner:
    def __init__(self, nc):
        import concourse.mybir as mybir
        import jax
        from jax.experimental.shard_map import shard_map
        from jax.sharding import Mesh, NamedSharding, PartitionSpec
        from concourse import bass2jax

        bass2jax.install_neuronx_cc_hook()
        self.jax = jax
        self.nc = nc

        part_name = nc.partition_id_tensor.name if nc.partition_id_tensor else None
        in_names, out_names, out_avals, zero_outs = [], [], [], []
        for alloc in nc.m.functions[0].allocations:
            if not isinstance(alloc, mybir.MemoryLocationSet):
                continue
            name = alloc.memorylocations[0].name
            if alloc.kind == "ExternalInput":
                if name != part_name:
                    in_names.append(name)
            elif alloc.kind == "ExternalOutput":
                out_names.append(name)
                shape = tuple(alloc.tensor_shape)
                dtype = mybir.dt.np(alloc.dtype)
                out_avals.append(jax.core.ShapedArray(shape, dtype))
                zero_outs.append(np.zeros(shape, dtype))
        self.in_names = in_names
        self.out_names = out_names
        self.out_avals = out_avals
        n_params = len(in_names)

        bind_names = in_names + out_names
        if part_name is not None:
            bind_names = bind_names + [part_name]

        def _body(*args):
            operands = list(args)
            if part_name is not None:
                operands.append(bass2jax.partition_id_tensor())
            outs = bass2jax._bass_exec_p.bind(
                *operands,
                out_avals=tuple(out_avals),
                in_names=tuple(bind_names),
                out_names=tuple(out_names),
                lowering_input_output_aliases=(),
                sim_require_finite=True,
                sim_require_nnan=True,
                nc=nc,
            )
            return tuple(outs)

        self._bind = _body
        devices = jax.devices()[:NCORES]
        self.mesh = Mesh(np.asarray(devices), ("core",))
        specs = (PartitionSpec("core"),) * (n_params + len(out_names))
        self.sharding = NamedSharding(self.mesh, PartitionSpec("core"))
        self.jitted = jax.jit(
            shard_map(
                _body, mesh=self.mesh,
                in_specs=specs,
                out_specs=(PartitionSpec("core"),) * len(out_names),
                check_rep=False,
            ),
            keep_unused=True,
        )
        self.dev_zeros = [
            jax.device_put(
                np.zeros((NCORES * z.shape[0], *z.shape[1:]), z.dtype),
                self.sharding,
            )
            for z in zero_outs
        ]
        self.dev_inputs = None
        self.dev_inputs_key = None

    def stage(self, in_maps, key=None):
        if key is not None and key == self.dev_inputs_key:
            return
        concat = [
            np.concatenate([in_maps[c][n] for c in range(NCORES)], axis=0)
            for n in self.in_names
        ]
        self.dev_inputs = [
            self.jax.device_put(a, self.sharding) for a in concat
        ]
        self.jax.block_until_ready(self.dev_inputs)
        self.dev_inputs_key = key

    def execute(self):
        out_arrs = self.jitted(*self.dev_inputs, *self.dev_zeros)
        self.jax.block_until_ready(out_arrs)
        return out_arrs

    def results(self, out_arrs):
        return [
            {
                n: np.asarray(out_arrs[i]).reshape(
                    NCORES, *self.out_avals[i].shape)[c]
                for i, n in enumerate(self.out_names)
            }
            for c in range(NCORES)
        ]


def _get_runner(inputs):
    in_maps = _prepare(inputs)
    if "r" not in _CACHE:
        _CACHE["r"] = _Runner(_build_program())
    runner = _CACHE["r"]
    key = tuple(id(inputs[k]) for k in sorted(inputs))
    runner.stage(in_maps, key=key)
    return runner


def run(inputs, trace=False):
    runner = _get_runner(inputs)
    out_arrs = runner.execute()
    res = runner.results(out_arrs)
    # core 2b owns tokens 0-511 of batch b, core 2b+1 owns 512-1023
    outs = np.stack([
        np.concatenate([res[2 * b]["out"], res[2 * b + 1]["out"]], axis=-1)
        for b in range(B)
    ])  # [4, 768, 1024]
    return outs.reshape(B, DIM, H, W).astype(np.float32), None


def time_device(inputs, iters=3):
    import time as _time

    runner = _get_runner(inputs)
    runner.execute()
    times = []
    for _ in range(iters):
        t0 = _time.perf_counter()
        runner.execute()
        times.append(_time.perf_counter() - t0)
    return times


def time_marginal(inputs, n=10):
    import time as _time

    runner = _get_runner(inputs)
    runner.execute()
    jax = runner.jax

    def run_n(k):
        t0 = _time.perf_counter()
        outs = None
        for _ in range(k):
            outs = runner.jitted(*runner.dev_inputs, *runner.dev_zeros)
        jax.block_until_ready(outs)
        return _time.perf_counter() - t0

    t_small = min(run_n(2) for _ in range(2))
    t_big = min(run_n(2 + n) for _ in range(2))
    return (t_big - t_small) / n


def kernel(**inputs):
    out, _ = run(inputs)
    return out
